# revision 58
# baseline (speedup 1.0000x reference)
"""GNN message-passing kernel for 8 Trainium2 NeuronCores.

Full (unsharded) inputs in, full output out. Data-parallel over the batch
dimension: 64 graphs -> 8 cores x 8 graphs. Parameters replicated.

Math per graph (reference semantics):
  F1 = sigmoid((Wv W) @ x^T)            [RP=64, N=512]   (WvW host-fused)
  Fn = F1
  repeat 7x (t=1..7):
    WwF = Ww @ Fn                       [64, 512]
    S   = Fn^T(r-contract) WwF + adjm   [512, 512], adjm = -25*(1-adj)
    em  = exp(S)          (masked entries underflow to ~e^-25 ~ 0)
    prop= [FnT | ones64] @ em -> [128, 512]; rows 64:128 = Z (softmax
          denominators, replicated 64x by the PE for free)
    Fn  = (prop[0:64] * F1) * recip(Z)
  gates q=0..7: g_q = 256 + 0.5*sum_n tanh(0.5*Wg Fn_q)  [64]
  fT = concat(gates); fT /= ||fT||; out = MLP(fT)        [128]

v7 restructure vs the 362us baseline (now ~272us):
  * adjacency mask folded ADDITIVELY into the S accumulation on the PE
    (identity-lhsT matmul of host-precomputed -25*(1-adj) bf16); exp
    underflows masked entries; no DVE mask multiply at all.
  * prop lhsT ones-columns at cols 0:64 -> the PE broadcasts the softmax
    denominator Z into PSUM partitions 0:64 for free (where the recip
    ucode can read it directly). Tail is zr=recip(pp[0:64]),
    fnu=pp[64:128]*F1 (mixed-base TT), fn=fnu*zr -- 3 DVE ops; the ACT
    Z-row copy, GPSIMD partition-broadcast and one DVE mult are gone.
  * graphs processed in pairs: even graph state (fn/f1/wwf) on SBUF
    partitions 0:64, odd on 64:128, so S and comb matmuls of the two
    graphs issue as row-tiled K=64 pairs in disjoint PE quadrants
    (concurrent fill; drains still share the PSUM write port).
  * one flat (t, pair) slot pipeline with tail at 1-slot stagger; paired
    tanh gate (one [128,N] activation + accum per pair per step); gate
    tail rebuilt via PE transpose + partition-crossing SBUF DMAs.
  * adjacency DMAs issue from the GPSIMD queue (ACT queue freed).
"""

import os
import time

import ml_dtypes
import numpy as np

import concourse.bacc as bacc
import concourse.tile as tile
import concourse.mybir as mybir
from concourse.bass_utils import run_bass_kernel_spmd

F32 = mybir.dt.float32
BF16 = mybir.dt.bfloat16
U32 = mybir.dt.uint32
AF = mybir.ActivationFunctionType
ALU = mybir.AluOpType

B, N, FEAT, EMB, RP = 64, 512, 256, 128, 64
T = 8          # MAX_WALK_LEN (1 initial gate + 7 propagation steps)
NCORES = 8
BPC = B // NCORES   # graphs per core
NCH = N // 128      # n-chunks of 128
D0 = RP * T         # 512, MLP width

_STATE = {}


def ts(i, size):
    return slice(i * size, (i + 1) * size)


def _build_program():
    nc = bacc.Bacc("TRN2", target_bir_lowering=False, debug=False,
                   num_devices=NCORES)
    p = {}
    p["xT"] = nc.dram_tensor("xT", [BPC, FEAT, N], BF16, kind="ExternalInput").ap()
    p["adjb"] = nc.dram_tensor("adjb", [BPC, N, N], BF16, kind="ExternalInput").ap()
    for name, shape, dt in [("wvwt", [FEAT, RP], BF16),
                            ("wgwwt", [RP, 2 * RP], BF16),
                            ("w0t", [D0, D0], F32), ("w1t", [D0, D0], F32),
                            ("w2t", [D0, D0 // 2], F32),
                            ("w3t", [D0 // 2, 128], F32),
                            ("b0", [D0], F32), ("b1", [D0], F32),
                            ("b2", [D0 // 2], F32), ("b3", [128], F32),
                            ("ident", [128, 128], F32),
                            ("identb", [128, 128], BF16)]:
        p[name] = nc.dram_tensor(name, shape, dt, kind="ExternalInput").ap()
    p["outT"] = nc.dram_tensor("outT", [128, BPC], F32, kind="ExternalOutput").ap()

    with tile.TileContext(nc) as tc:
        import contextlib
        with contextlib.ExitStack() as ctx:
            const = ctx.enter_context(tc.tile_pool(name="const", bufs=1))
            p["wvwt_s"] = const.tile([128, 2 * 64], BF16, tag="wvwt", name="wvwt_s")
            p["wgwwt_s"] = const.tile([128, 128], BF16, tag="wgwwt", name="wgwwt_s")
            p["ident_s"] = const.tile([128, 128], F32, tag="ident", name="ident_s")
            p["identb_s"] = const.tile([128, 128], BF16, tag="identb",
                                       name="identb_s")
            p["w0t_s"] = const.tile([128, 4 * D0], F32, tag="w0t", name="w0t_s")
            p["w1t_s"] = const.tile([128, 4 * D0], F32, tag="w1t", name="w1t_s")
            p["w2t_s"] = const.tile([128, 4 * (D0 // 2)], F32, tag="w2t",
                                    name="w2t_s")
            p["w3t_s"] = const.tile([128, 2 * 128], F32, tag="w3t", name="w3t_s")
            p["b0_s"] = const.tile([128, 4], F32, tag="b0", name="b0_s")
            p["b1_s"] = const.tile([128, 4], F32, tag="b1", name="b1_s")
            p["b2_s"] = const.tile([128, 2], F32, tag="b2", name="b2_s")
            p["b3_s"] = const.tile([128, 1], F32, tag="b3", name="b3_s")
            # paired gate accumulators: gp[pr][r + 64*gg, q] = gate sum of
            # graph 2*pr+gg, walk step q, row r
            gp = [const.tile([128, T], F32, tag=f"gp{i}", name=f"gp{i}")
                  for i in range(BPC // 2)]
            # per-graph persistent state
            adj_g = [const.tile([128, NCH * N], BF16, tag=f"adj{g}",
                                name=f"adj{g}") for g in range(BPC)]
            # fn/f1/wwf are [128, N]; even graphs live on rows 0:64, odd
            # graphs on rows 64:128, so the two graphs of a pair can run
            # K=64 matmuls CONCURRENTLY in disjoint row-groups of the PE
            f1_g = [const.tile([128, N], BF16, tag=f"f1_{g}", name=f"f1_{g}")
                    for g in range(BPC)]
            f1f_g = [const.tile([64, N], F32, tag=f"f1f{g}", name=f"f1f{g}")
                     for g in range(BPC)]
            fn_g = [const.tile([128, N], BF16, tag=f"fn{g}", name=f"fn{g}")
                    for g in range(BPC)]
            wwf_g = [const.tile([128, N], BF16, tag=f"wwf{g}", name=f"wwf{g}")
                     for g in range(BPC)]
            fnt_g = [const.tile([128, NCH * 128], BF16, tag=f"fnt{g}",
                                name=f"fnt{g}") for g in range(BPC)]

            nc.sync.dma_start(p["wvwt_s"][:, 0:64], p["wvwt"][0:128, :])
            nc.sync.dma_start(p["wvwt_s"][:, 64:128], p["wvwt"][128:256, :])
            nc.sync.dma_start(p["wgwwt_s"][0:64, :], p["wgwwt"][:, :])
            nc.sync.dma_start(p["wgwwt_s"][64:128, :], p["wgwwt"][:, :])
            nc.sync.dma_start(p["ident_s"][:], p["ident"][:, :])
            nc.sync.dma_start(p["identb_s"][:], p["identb"][:, :])
            for g in range(BPC):
                # cols j*128 .. j*128+64 of each chunk hold ones: the prop
                # matmul then replicates Z into PSUM partitions 0:64 (where
                # the recip ucode can read it); fnT sits at cols 64:128 so
                # prop rows land on partitions 64:128.
                nc.gpsimd.memset(fnt_g[g][:], 0.0)
                for j in range(NCH):
                    nc.gpsimd.memset(fnt_g[g][:, j * 128:j * 128 + 64], 1.0)

            # transient pools
            p["xg"] = ctx.enter_context(tc.tile_pool(name="xg", bufs=2))
            p["e"] = ctx.enter_context(tc.tile_pool(name="e", bufs=8))
            p["scr"] = ctx.enter_context(tc.tile_pool(name="scr", bufs=3))
            p["zcr"] = ctx.enter_context(tc.tile_pool(name="zcr", bufs=3))
            p["fnu"] = ctx.enter_context(tc.tile_pool(name="fnu", bufs=3))
            p["frow"] = ctx.enter_context(tc.tile_pool(name="frow", bufs=1))
            p["tiny"] = ctx.enter_context(tc.tile_pool(name="tiny", bufs=1))
            p["mlp"] = ctx.enter_context(tc.tile_pool(name="mlp", bufs=1))
            # PSUM: s_ps 2 bufs x 2 banks + pp 4 bufs x 1 bank = 8 banks
            p["s_ps"] = ctx.enter_context(
                tc.tile_pool(name="s_ps", bufs=2, space="PSUM"))
            p["pp"] = ctx.enter_context(
                tc.tile_pool(name="pp", bufs=4, space="PSUM"))

            def comb_pair(g0, g1, q, fn0, fn1, last=False):
                """row+col-tiled concurrent matmul pair on (fn0 rows 0:64,
                fn1 rows 64:128) -> one paired tanh + per-graph wwf."""
                pr = g0 // 2
                cbp = p["pp"].tile([128, N], F32, tag="pp", name="pp_t")
                nc.tensor.matmul(cbp[0:64, :], p["wgwwt_s"][0:64, 0:64],
                                 fn0[0:64, :], start=True, stop=True,
                                 tile_position=(0, 0), skip_group_check=True)
                nc.tensor.matmul(cbp[64:128, :], p["wgwwt_s"][64:128, 0:64],
                                 fn1[64:128, :], start=True, stop=True,
                                 tile_position=(64, 64),
                                 skip_group_check=True)
                if not last:
                    cww = p["pp"].tile([128, N], F32, tag="pp", name="pp_t")
                    nc.tensor.matmul(cww[0:64, :], p["wgwwt_s"][0:64, 64:128],
                                     fn0[0:64, :], start=True, stop=True,
                                     tile_position=(0, 0),
                                     skip_group_check=True)
                    nc.tensor.matmul(cww[64:128, :],
                                     p["wgwwt_s"][64:128, 64:128],
                                     fn1[64:128, :], start=True, stop=True,
                                     tile_position=(64, 64),
                                     skip_group_check=True)
                scr = p["scr"].tile([128, N], BF16)
                nc.scalar.activation(scr[:], cbp[:], AF.Tanh, scale=0.5,
                                     accum_out=gp[pr][:, q:q + 1])
                if not last:
                    nc.vector.tensor_copy(wwf_g[g0][0:64, :], cww[0:64, :])
                    nc.vector.tensor_copy(wwf_g[g1][64:128, :],
                                          cww[64:128, :])

            def transp(g, t):
                fn_cur = f1_g[g] if t == 1 else fn_g[g]
                lo = 64 * (g % 2)
                dst = fnt_g[g][:].rearrange(
                    "p (c k) -> p c k", k=128)[:, :, 64:128]
                nc.sync.dma_start_transpose(dst, fn_cur[lo:lo + 64, :])

            # ---------------- phase A ----------------
            for g in range(BPC):
                xg = p["xg"].tile([128, 2 * N], BF16)
                nc.sync.dma_start(
                    xg[:].rearrange("p (k m) -> p k m", m=N),
                    p["xT"][g].rearrange("(k p) m -> p k m", p=128))
                nc.gpsimd.dma_start(
                    adj_g[g][:].rearrange("p (c m) -> p c m", m=N),
                    p["adjb"][g].rearrange("(c p) m -> p c m", p=128))
                f1_ps = p["pp"].tile([64, N], F32, tag="pp", name="pp_t")
                for k in range(2):
                    nc.tensor.matmul(f1_ps[:], p["wvwt_s"][:, ts(k, 64)],
                                     xg[:, ts(k, N)], start=(k == 0),
                                     stop=(k == 1))
                scr = p["scr"].tile([64, N], F32)
                nc.scalar.activation(scr[:], f1_ps[:], AF.Tanh, scale=0.5)
                nc.vector.tensor_scalar(f1f_g[g][:], scr[:], 0.5, 0.5,
                                        ALU.mult, ALU.add)
                lo = 64 * (g % 2)
                nc.vector.tensor_copy(f1_g[g][lo:lo + 64, :], f1f_g[g][:])
                if g % 2 == 1:
                    comb_pair(g - 1, g, 0, f1_g[g - 1], f1_g[g])

            # MLP weights: load on sync queue once phase A issue is done
            def load_mlp_weights():
                for name, dst, d in [("w0t", p["w0t_s"], D0),
                                     ("w1t", p["w1t_s"], D0),
                                     ("w2t", p["w2t_s"], D0 // 2),
                                     ("w3t", p["w3t_s"], 128)]:
                    nc.sync.dma_start(
                        dst[:].rearrange("p (k d) -> p k d", d=d),
                        p[name].rearrange("(k p) d -> p k d", p=128))
                for name, dst, kch in [("b0", p["b0_s"], 4),
                                       ("b1", p["b1_s"], 4),
                                       ("b2", p["b2_s"], 2),
                                       ("b3", p["b3_s"], 1)]:
                    nc.sync.dma_start(
                        dst[:, 0:kch],
                        p[name].rearrange("(k p) -> p k", p=128))

            load_mlp_weights()

            # ---------------- main loop: software-pipelined wavefront -------
            # One flat slot stream over (t, g): slot i emits head(g,t) and
            # the tail+comb+next-transpose for slot i-2 (crossing step
            # boundaries), so the PE queue never drains and HAM stays warm.
            em_h = {}

            def pair_head(pr, t):
                g0, g1 = 2 * pr, 2 * pr + 1
                fn0 = f1_g[g0] if t == 1 else fn_g[g0]
                fn1 = f1_g[g1] if t == 1 else fn_g[g1]
                # per n-chunk: concurrent row-tiled S pair (g0 rows 0:64,
                # g1 rows 64:128) into one 2-bank tile, masks, one exp
                for j in range(NCH):
                    s_pj = p["s_ps"].tile([128, 2 * N], F32, tag="s",
                                          name="s_t")
                    nc.tensor.matmul(s_pj[:, 0:N],
                                     fn0[0:64, ts(j, 128)],
                                     wwf_g[g0][0:64, :], start=True,
                                     stop=False, tile_position=(0, 0),
                                     skip_group_check=True)
                    nc.tensor.matmul(s_pj[:, N:2 * N],
                                     fn1[64:128, ts(j, 128)],
                                     wwf_g[g1][64:128, :], start=True,
                                     stop=False, tile_position=(64, 0),
                                     skip_group_check=True)
                    # additive adjacency mask: s += I.T @ (-25*(1-adj))
                    nc.tensor.matmul(s_pj[:, 0:N], p["identb_s"][:, :],
                                     adj_g[g0][:, ts(j, N)], start=False,
                                     stop=True, skip_group_check=True)
                    nc.tensor.matmul(s_pj[:, N:2 * N], p["identb_s"][:, :],
                                     adj_g[g1][:, ts(j, N)], start=False,
                                     stop=True, skip_group_check=True)
                    e_pj = p["e"].tile([128, 2 * N], BF16)
                    nc.scalar.activation(e_pj[:], s_pj[:], AF.Exp)
                    em_h[(pr, j)] = e_pj

            def prop_tail(g, t):
                pr, half = g // 2, (g % 2) * N
                pp = p["pp"].tile([128, N], F32, tag="pp", name="pp_t")
                for jj in range(NCH):
                    nc.tensor.matmul(pp[:], fnt_g[g][:, ts(jj, 128)],
                                     em_h[(pr, jj)][:, half:half + N],
                                     start=(jj == 0), stop=(jj == 3),
                                     skip_group_check=True)
                # Z replicated on partitions 0:64, prop rows on 64:128
                zr = p["zcr"].tile([64, N], F32, tag="zr")
                nc.vector.reciprocal_approx_fast(zr[:], pp[0:64, :])
                fnu = p["fnu"].tile([64, N], F32)
                nc.vector.tensor_tensor(fnu[:], pp[64:128, :], f1f_g[g][:],
                                        ALU.mult)
                lo = 64 * (g % 2)
                nc.vector.tensor_tensor(fn_g[g][lo:lo + 64, :], fnu[:], zr[:],
                                        ALU.mult)

            slots = [(t, pr) for t in range(1, T) for pr in range(BPC // 2)]
            for g in range(BPC):
                transp(g, 1)

            def slot_tail(t, pr):
                g0, g1 = 2 * pr, 2 * pr + 1
                prop_tail(g0, t)
                prop_tail(g1, t)
                if t + 1 < T:
                    transp(g0, t + 1)
                    transp(g1, t + 1)
                comb_pair(g0, g1, t, fn_g[g0], fn_g[g1], last=(t == T - 1))

            for i, (t, pr) in enumerate(slots):
                pair_head(pr, t)
                if i >= 1:
                    slot_tail(*slots[i - 1])
            slot_tail(*slots[-1])

            # ---------------- gate affine + L2 norm + MLP ----------------
            # gp[pr] is [128, T] with graph 2pr on partitions 0:64 and
            # 2pr+1 on 64:128. PE-transpose to [T, 128], affine on evac,
            # then partition->free DMAs build f_row[b, 64q+r].
            ident = p["ident_s"]
            f_row = p["frow"].tile([BPC, N], F32)
            for pr in range(BPC // 2):
                t_ps = p["pp"].tile([T, 128], F32, tag="pp", name="pp_t")
                nc.tensor.transpose(t_ps[:], gp[pr][:, 0:T], ident[:, 0:128])
                gq = p["scr"].tile([T, 128], F32)
                nc.vector.tensor_scalar(gq[:], t_ps[:], 0.5, 256.0,
                                        ALU.mult, ALU.add)
                for gg in range(2):
                    eng = nc.sync if gg == 0 else nc.gpsimd
                    eng.dma_start(
                        f_row[2 * pr + gg:2 * pr + gg + 1, :].rearrange(
                            "o (q r) -> o q r", r=64),
                        gq[0:T, 64 * gg:64 * gg + 64])

            sq = p["frow"].tile([BPC, N], F32)
            ss = p["tiny"].tile([BPC, 1], F32, tag="ss")
            nc.scalar.activation(sq[:], f_row[:], AF.Square, accum_out=ss[:])

            tmp = p["tiny"].tile([BPC, 1], F32, tag="t0")
            y = p["tiny"].tile([BPC, 1], F32, tag="t1")
            a = p["tiny"].tile([BPC, 1], F32, tag="t2")
            nc.vector.tensor_scalar(tmp[:].bitcast(U32), ss[:].bitcast(U32),
                                    1, None, ALU.logical_shift_right)
            nc.vector.tensor_scalar(tmp[:].bitcast(U32), tmp[:].bitcast(U32),
                                    0x1FBD1DF5, None, ALU.add)
            nc.vector.reciprocal_approx_fast(y[:], tmp[:])
            for _ in range(3):
                nc.vector.tensor_tensor(a[:], y[:], y[:], ALU.mult)
                nc.vector.tensor_tensor(a[:], ss[:], a[:], ALU.mult)
                nc.vector.tensor_scalar(a[:], a[:], -0.5, 1.5, ALU.mult, ALU.add)
                nc.vector.tensor_tensor(y[:], y[:], a[:], ALU.mult)

            fn_row = p["frow"].tile([BPC, N], F32)
            nc.vector.tensor_scalar(fn_row[:], f_row[:], y[:], None, ALU.mult)

            h0 = p["mlp"].tile([128, 4 * BPC], F32, tag="h0")
            for j in range(NCH):
                t_ps = p["pp"].tile([128, BPC], F32, tag="pp", name="pp_t")
                nc.tensor.transpose(t_ps[:, 0:BPC], fn_row[:, ts(j, 128)],
                                    ident[0:BPC, 0:BPC])
                nc.vector.tensor_copy(h0[:, ts(j, BPC)], t_ps[:, 0:BPC])

            def layer(h_in, kch, jch, w_s, b_s, act, tag):
                h_out = p["mlp"].tile([128, jch * BPC], F32, tag=tag)
                for j in range(jch):
                    mm = p["pp"].tile([128, BPC], F32, tag="pp", name="pp_t")
                    for k in range(kch):
                        nc.tensor.matmul(mm[:], w_s[:, k * (jch * 128) + j * 128:
                                                     k * (jch * 128) + (j + 1) * 128],
                                         h_in[:, ts(k, BPC)],
                                         start=(k == 0), stop=(k == kch - 1))
                    nc.scalar.activation(h_out[:, ts(j, BPC)], mm[:], act,
                                         bias=b_s[:, j:j + 1])
                return h_out

            h1 = layer(h0, 4, 4, p["w0t_s"], p["b0_s"], AF.Relu, "h1")
            h2 = layer(h1, 4, 4, p["w1t_s"], p["b1_s"], AF.Relu, "h2")
            h3 = layer(h2, 4, 2, p["w2t_s"], p["b2_s"], AF.Relu, "h3")
            h4 = layer(h3, 2, 1, p["w3t_s"], p["b3_s"], AF.Identity, "h4")
            nc.sync.dma_start(p["outT"][:, :], h4[:, 0:BPC])

    nc.compile()
    return nc


def _prep_inputs(inputs):
    bf = ml_dtypes.bfloat16
    x = np.asarray(inputs["node_attribute_matrix"], np.float32)
    adj = np.asarray(inputs["adjacent_matrix"])
    adjm = np.ascontiguousarray((-25.0 * (adj == 0)).astype(bf))
    xT = np.ascontiguousarray(x.transpose(0, 2, 1).astype(bf))  # [B, FEAT, N]
    wvw = (np.asarray(inputs["Wv"], np.float32)
           @ np.asarray(inputs["W"], np.float32))               # [RP, FEAT]

    common = {
        "wvwt": np.ascontiguousarray(wvw.T.astype(bf)),
        "wgwwt": np.ascontiguousarray(np.hstack([
            np.asarray(inputs["Wg"], np.float32).T,
            np.asarray(inputs["Ww"], np.float32).T]).astype(bf)),
        "w0t": np.ascontiguousarray(np.asarray(inputs["W0"], np.float32).T),
        "w1t": np.ascontiguousarray(np.asarray(inputs["W1"], np.float32).T),
        "w2t": np.ascontiguousarray(np.asarray(inputs["W2"], np.float32).T),
        "w3t": np.ascontiguousarray(np.asarray(inputs["W3"], np.float32).T),
        "b0": np.asarray(inputs["b0"], np.float32),
        "b1": np.asarray(inputs["b1"], np.float32),
        "b2": np.asarray(inputs["b2"], np.float32),
        "b3": np.asarray(inputs["b3"], np.float32),
        "ident": np.eye(128, dtype=np.float32),
        "identb": np.eye(128).astype(bf),
    }
    in_maps = []
    for c in range(NCORES):
        sl = slice(c * BPC, (c + 1) * BPC)
        m = dict(common)
        m["xT"] = xT[sl]
        m["adjb"] = adjm[sl]
        in_maps.append(m)
    return in_maps


def kernel(**inputs) -> np.ndarray:
    if "nc" not in _STATE:
        _STATE["nc"] = _build_program()
    nc = _STATE["nc"]
    in_maps = _prep_inputs(inputs)

    trace = bool(int(os.environ.get("GNN_TRACE", "0")))
    kwargs = {}
    if trace:
        kwargs = dict(trace=True, tmpdir=os.environ.get("GNN_TRACE_DIR") or None)
    t0 = time.time()
    res = run_bass_kernel_spmd(nc, in_maps, list(range(NCORES)), **kwargs)
    _STATE["wall_s"] = time.time() - t0
    _STATE["exec_time_ns"] = res.exec_time_ns
    _STATE["results"] = res

    out = np.empty((B, 128), np.float32)
    for c in range(NCORES):
        out[c * BPC:(c + 1) * BPC] = res.results[c]["outT"].T
    return out
